# revision 5
# baseline (speedup 1.0000x reference)
# Trainium2 Bass kernel for nn_LiteMultiscaleAttention (8-core data-parallel over batch).
#
# Fused ct-streaming pipeline per core (one batch):
#   for each of 12 channel tiles (ct):
#     qkv[ct] = Wqkv @ x                    (4 k-tile matmuls x 8 spatial chunks)
#     agg[ct] = fused dw5x5+grouped-pw      (25-tap block-diag matmuls, PSUM accum)
#     route padded 16-row blocks (contiguous DMA) to attention holds / q scratch
#     every 3rd ct: stage A (vk^T accumulation) for one qkv-head group and one
#       agg-head group, interleaved into the matmul stream to keep the PE warm
#   stage B + proj fused nt-outer: normalize attention chunk -> proj -> BN -> y
import sys
import os
import numpy as np

sys.path.insert(0, '/opt/trn_rl_repo')

import ml_dtypes
import concourse.bass as bass
import concourse.mybir as mybir
import concourse.tile as tile
from concourse import bacc
from concourse.bass_utils import run_bass_kernel_spmd
from concourse.masks import make_identity

BF16 = mybir.dt.bfloat16
F32 = mybir.dt.float32

B, CIN, H, W = 8, 512, 64, 64
S = H * W                 # 4096
SP = 68 * 68              # padded spatial 4624
C3 = 1536
NCT = 12
NKT = 4
NNT = 8
EPS = 1e-15
BN_EPS = 1e-5

_CACHED = {}


def build_program():
    nc = bacc.Bacc('TRN2', target_bir_lowering=False, debug=False)

    # ---------------- DRAM I/O ----------------
    x_d = nc.dram_tensor('x16', [128, NKT, S], BF16, kind='ExternalInput')
    wq_d = nc.dram_tensor('wq', [128, NKT, C3], BF16, kind='ExternalInput')
    w2c_d = nc.dram_tensor('w2c', [128, 25, NCT, 32], BF16, kind='ExternalInput')
    wp_d = nc.dram_tensor('wp', [128, 8, 512], BF16, kind='ExternalInput')
    bnb_d = nc.dram_tensor('bnb', [128, 4], F32, kind='ExternalInput')
    obd_d = nc.dram_tensor('obd', [128, 64], BF16, kind='ExternalInput')
    y_d = nc.dram_tensor('y_b', [512, S], F32, kind='ExternalOutput')
    qst_d = nc.dram_tensor('q_stack', [128, 8, S], BF16)

    with tile.TileContext(nc) as tc:
        from contextlib import ExitStack
        ctx = ExitStack()
        with ctx:
            stat = ctx.enter_context(tc.tile_pool(name='stat', bufs=1))

            id128 = stat.tile([128, 128], BF16)
            make_identity(nc, id128[:])
            ones_col = stat.tile([128, 1], BF16)
            nc.gpsimd.memset(ones_col[:], 1.0)
            eps_col = stat.tile([128, 1], F32)
            nc.gpsimd.memset(eps_col[:], EPS)
            ones64 = stat.tile([128, 64], BF16)
            nc.sync.dma_start(ones64[:], obd_d.ap())

            # 128-wide block-diag attention weights (built by stage A evacuations)
            bdn = stat.tile([128, 8, 128], BF16)
            nc.gpsimd.memset(bdn[:], 0.0)
            bdd = stat.tile([128, 8, 128], BF16)
            nc.gpsimd.memset(bdd[:], 0.0)
            den_col = stat.tile([128, 8, 1], F32)

            qbap = ctx.enter_context(tc.tile_pool(name='qbap', bufs=1))
            qbA = qbap.tile([128, 2, S], BF16)

            es2 = ExitStack()
            w1 = es2.enter_context(tc.tile_pool(name='w1', bufs=1))
            wq = w1.tile([128, NKT, C3], BF16)
            nc.sync.dma_start(wq[:], wq_d.ap())
            w2p = es2.enter_context(tc.tile_pool(name='w2p', bufs=1))
            w2c = w2p.tile([128, 25, NCT, 32], BF16)
            nc.sync.dma_start(w2c[:], w2c_d.ap())
            x16p = es2.enter_context(tc.tile_pool(name='x16p', bufs=1))
            x16 = x16p.tile([128, NKT, S], BF16)
            for kt in range(NKT):
                nc.sync.dma_start(x16[:, kt, :], x_d.ap()[:, kt, :])

            qkvp = es2.enter_context(tc.tile_pool(name='qkvp', bufs=1))
            astp = es2.enter_context(tc.tile_pool(name='astp', bufs=1))
            psum = es2.enter_context(tc.tile_pool(name='psum', bufs=1, space='PSUM'))
            holdp = es2.enter_context(tc.tile_pool(name='holdp', bufs=2))
            lhsp = es2.enter_context(tc.tile_pool(name='lhsp', bufs=2))
            trp = es2.enter_context(tc.tile_pool(name='trp', bufs=4))

            # persistent padded tiles: pads zeroed once, interiors rewritten per ct
            qkvt = [qkvp.tile([128, 68, 68], BF16, name=f'qkvt{j}') for j in range(2)]
            asts = [astp.tile([128, 68, 68], BF16, name=f'ast{j}') for j in range(2)]
            for t in qkvt + asts:
                nc.gpsimd.memset(t[:], 0.0)

            vones_init = [True]

            def stage_a_for_group(tg, khold, vhold):
                psA = psum.tile([128, 129], F32, tag='psA', name=f'psA{tg}')
                nst = S // 128
                for st in range(nst):
                    psT = psum.tile([128, 128], BF16, tag='tr', bufs=2, name=f'psT{tg}_{st}')
                    kTt = trp.tile([128, 128], BF16, tag='kTt', name=f'kTt{tg}_{st}')
                    nc.tensor.transpose(psT[:], khold[:, 128 * st:128 * (st + 1)], id128[:])
                    if st % 2 == 0:
                        nc.vector.tensor_copy(kTt[:], psT[:])
                    else:
                        nc.scalar.activation(kTt[:], psT[:], mybir.ActivationFunctionType.Copy)
                    psT2 = psum.tile([128, 128], BF16, tag='tr', bufs=2, name=f'psU{tg}_{st}')
                    vTt = trp.tile([128, 129], BF16, tag='vTt', name=f'vTt{tg}_{st}')
                    # ones column: written once per physical buffer (4-buf rotation),
                    # evacuations below only touch cols 0:128
                    if vones_init[0] and st < 4:
                        nc.vector.memset(vTt[:, 128:129], 1.0)
                    nc.tensor.transpose(psT2[:], vhold[:, 128 * st:128 * (st + 1)], id128[:])
                    if st % 2 == 1:
                        nc.vector.tensor_copy(vTt[:, 0:128], psT2[:])
                    else:
                        nc.scalar.activation(vTt[:, 0:128], psT2[:],
                                             mybir.ActivationFunctionType.Copy)
                    nc.tensor.matmul(psA[:], kTt[:], vTt[:],
                                     start=(st == 0), stop=(st == nst - 1))
                vones_init[0] = False
                # evacuate diag blocks masked by the block-diag ones pattern
                for j in range(4):
                    r0 = 32 * j
                    cc = 32 * (j % 2)
                    nc.vector.scalar_tensor_tensor(
                        bdn[r0:r0 + 32, tg, r0:r0 + 32],
                        psA[r0:r0 + 32, r0:r0 + 32], 1.0,
                        ones64[r0:r0 + 32, cc:cc + 32],
                        mybir.AluOpType.mult, mybir.AluOpType.mult)
                nc.vector.tensor_copy(den_col[:, tg, :], psA[:, 128:129])
                for j in range(4):
                    r0 = 32 * j
                    cc = 32 * (j % 2)
                    nc.vector.tensor_scalar_mul(
                        bdd[r0:r0 + 32, tg, r0:r0 + 32],
                        ones64[r0:r0 + 32, cc:cc + 32],
                        den_col[r0:r0 + 32, tg, :])

            def route(ct, agg_half, src_tile, kdst, vdst):
                eng = [nc.scalar, nc.sync]
                for bi in range(8):
                    c = 128 * ct + 16 * bi
                    h = (32 if agg_half else 0) + c // 48
                    r = c % 48
                    tg = h // 8
                    hl = h % 8
                    src = src_tile[16 * bi:16 * bi + 16, 2:66, 2:66]
                    e = eng[bi % 2]
                    if r == 0:
                        e.dma_start(qst_d.ap()[16 * hl:16 * hl + 16, tg, :], src)
                    elif r == 16:
                        e.dma_start(kdst[16 * hl:16 * hl + 16, :], src)
                    else:
                        e.dma_start(vdst[16 * hl:16 * hl + 16, :], src)

            # ---------------- main ct loop ----------------
            holds = {}
            for ct in range(NCT):
                if ct % 3 == 0:
                    for nm in ('kh', 'vh', 'akh', 'avh'):
                        hti = holdp.tile([128, S], BF16, tag=nm, name=f'h_{nm}_{ct}')
                        holds[nm] = hti
                qt = qkvt[ct % 2]
                # qkv for this ct
                for nt in range(NNT):
                    ps = psum.tile([128, 512], F32, tag=f'q{nt % 2}', name=f'q{ct}_{nt}')
                    for kt in range(NKT):
                        nc.tensor.matmul(
                            ps[:], wq[:, kt, 128 * ct:128 * (ct + 1)],
                            x16[:, kt, 512 * nt:512 * (nt + 1)],
                            start=(kt == 0), stop=(kt == NKT - 1))
                    dst = qt[:, 8 * nt + 2:8 * nt + 10, 2:66]
                    if nt % 2 == 0:
                        nc.vector.tensor_copy(dst, ps[:])
                    else:
                        nc.scalar.activation(dst, ps[:], mybir.ActivationFunctionType.Copy)

                route(ct, False, qt, holds['kh'], holds['vh'])

                if ct % 3 == 2:
                    nc.vector.tensor_scalar_max(holds['kh'][:], holds['kh'][:], 0.0)
                    stage_a_for_group(ct // 3, holds['kh'], holds['vh'])

                # agg: expand compact W2 into block-diag lhsT (diag blocks only;
                # off-diag stays zero from the initial memset)
                lt = lhsp.tile([128, 25, 128], BF16, tag='lt', name=f'lt{ct}')
                if ct < 2:
                    nc.gpsimd.memset(lt[:], 0.0)
                for blk in range(4):
                    nc.gpsimd.dma_start(lt[32 * blk:32 * blk + 32, :, 32 * blk:32 * blk + 32],
                                        w2c[32 * blk:32 * blk + 32, :, ct, :])
                at = asts[ct % 2]
                for nt in range(NNT):
                    ps = psum.tile([128, 512], F32, tag=f'a{nt % 2}', name=f'ag{ct}_{nt}')
                    for tap in range(25):
                        dy, dx = tap // 5, tap % 5
                        rhs = qt[:, 8 * nt + dy:8 * nt + dy + 8, dx:dx + 64]
                        nc.tensor.matmul(ps[:], lt[:, tap, :], rhs,
                                         start=(tap == 0), stop=(tap == 24))
                    dst = at[:, 8 * nt + 2:8 * nt + 10, 2:66]
                    if nt % 2 == 0:
                        nc.vector.tensor_copy(dst, ps[:])
                    else:
                        nc.scalar.activation(dst, ps[:], mybir.ActivationFunctionType.Copy)

                route(ct, True, at, holds['akh'], holds['avh'])

                if ct % 3 == 2:
                    nc.vector.tensor_scalar_max(holds['akh'][:], holds['akh'][:], 0.0)
                    stage_a_for_group(4 + ct // 3, holds['akh'], holds['avh'])
                    if ct == 2:
                        for sl, tg in ((0, 0), (1, 4)):
                            nc.sync.dma_start(qbA[:, sl, :], qst_d.ap()[:, tg, :])
                        nc.vector.tensor_scalar_max(qbA[:, 0, :], qbA[:, 0, :], 0.0)
                        nc.scalar.activation(qbA[:, 1, :], qbA[:, 1, :],
                                             mybir.ActivationFunctionType.Relu)

            es2.close()

            # ---------------- stage B + proj fused (nt-outer) ----------------
            psumB = ctx.enter_context(tc.tile_pool(name='psumB', bufs=2, space='PSUM'))
            wpp = ctx.enter_context(tc.tile_pool(name='wpp', bufs=1))
            wp = wpp.tile([128, 8, 512], BF16)
            nc.sync.dma_start(wp[:], wp_d.ap())
            bnbp = ctx.enter_context(tc.tile_pool(name='bnbp', bufs=1))
            bnb = bnbp.tile([128, 4], F32)
            nc.sync.dma_start(bnb[:], bnb_d.ap())
            qbp = ctx.enter_context(tc.tile_pool(name='qbp', bufs=1))
            qb = qbp.tile([128, 6, S], BF16)
            late = [1, 2, 3, 5, 6, 7]
            for sl, tg in enumerate(late):
                e = nc.sync if sl % 2 == 0 else nc.scalar
                e.dma_start(qb[:, sl, :], qst_d.ap()[:, tg, :])
            for sl, tg in enumerate(late):
                if sl % 2 == 0:
                    nc.vector.tensor_scalar_max(qb[:, sl, :], qb[:, sl, :], 0.0)
                else:
                    nc.scalar.activation(qb[:, sl, :], qb[:, sl, :],
                                         mybir.ActivationFunctionType.Relu)
            qslot = {0: (qbA, 0), 4: (qbA, 1)}
            for sl, tg in enumerate(late):
                qslot[tg] = (qb, sl)

            attnp = ctx.enter_context(tc.tile_pool(name='attnp', bufs=3))
            nump = ctx.enter_context(tc.tile_pool(name='nump', bufs=2))
            drp = ctx.enter_context(tc.tile_pool(name='drp', bufs=2))
            ysp = ctx.enter_context(tc.tile_pool(name='ysp', bufs=3))
            for nt in range(NNT):
                psPs = [psumB.tile([128, 512], F32, tag=f'psP{mt}', bufs=1,
                                   name=f'pp{nt}_{mt}') for mt in range(4)]
                for tg in range(8):
                    qtile, sl = qslot[tg]
                    win = qtile[:, sl, 512 * nt:512 * (nt + 1)]
                    psN = psumB.tile([128, 512], F32, tag='psN', bufs=2, name=f'psN{nt}_{tg}')
                    nc.tensor.matmul(psN[:], bdn[:, tg, :], win, start=True, stop=True)
                    psD = psumB.tile([128, 512], F32, tag='psD', bufs=2, name=f'psD{nt}_{tg}')
                    nc.tensor.matmul(psD[:], bdd[:, tg, :], win, start=True, stop=True)
                    den = nump.tile([128, 512], F32, tag='den', name=f'den{nt}_{tg}')
                    nc.scalar.activation(den[:], psD[:], mybir.ActivationFunctionType.Identity,
                                         bias=eps_col[:, 0:1])
                    drt = drp.tile([128, 512], F32, tag='drt', name=f'drt{nt}_{tg}')
                    nc.vector.reciprocal_approx_fast(drt[:], den[:])
                    attc = attnp.tile([128, 512], BF16, tag='attc', name=f'at{nt}_{tg}')
                    nc.vector.scalar_tensor_tensor(
                        attc[:], psN[:], 1.0, drt[:],
                        mybir.AluOpType.mult, mybir.AluOpType.mult)
                    for mt in range(4):
                        nc.tensor.matmul(psPs[mt][:], wp[:, tg, 128 * mt:128 * (mt + 1)],
                                         attc[:], start=(tg == 0), stop=(tg == 7))
                for mt in range(4):
                    ys = ysp.tile([128, 512], F32, tag='ys', name=f'ys{nt}_{mt}')
                    if mt % 2 == 0:
                        nc.vector.tensor_scalar_add(ys[:], psPs[mt][:], bnb[:, mt:mt + 1])
                    else:
                        nc.scalar.activation(ys[:], psPs[mt][:],
                                             mybir.ActivationFunctionType.Identity,
                                             bias=bnb[:, mt:mt + 1])
                    nc.sync.dma_start(
                        y_d.ap()[128 * mt:128 * (mt + 1), 512 * nt:512 * (nt + 1)], ys[:])

    nc.compile()
    return nc


def host_weights(w_qkv, w_dw, w_pw, w_proj, bn_gamma, bn_beta, bn_mean, bn_var):
    wq = w_qkv[:, :, 0, 0].astype(np.float32)       # [1536, 512]
    wdw = w_dw[:, 0].reshape(1536, 25).astype(np.float32)
    wpw = w_pw[:, :, 0, 0].astype(np.float32)       # [1536, 32]
    A = wdw.reshape(12, 4, 32, 25)
    Bm = wpw.reshape(12, 4, 32, 32)
    W2c = A[:, :, :, :, None] * Bm.transpose(0, 1, 3, 2)[:, :, :, None, :]
    W2c = np.ascontiguousarray(W2c.transpose(1, 2, 3, 0, 4).reshape(128, 25, 12, 32))
    inv = bn_gamma / np.sqrt(bn_var + BN_EPS)
    wp_f = (w_proj[:, :, 0, 0] * inv[:, None]).T    # [1024, 512] lhsT
    bnb = (bn_beta - bn_mean * inv).astype(np.float32)

    wq_dev = np.ascontiguousarray(
        wq.T.reshape(NKT, 128, C3).transpose(1, 0, 2)).astype(ml_dtypes.bfloat16)
    w2c_dev = W2c.astype(ml_dtypes.bfloat16)
    wp_dev = np.ascontiguousarray(
        wp_f.reshape(8, 128, 512).transpose(1, 0, 2)).astype(ml_dtypes.bfloat16)
    bnb_dev = np.ascontiguousarray(bnb.reshape(4, 128).T).astype(np.float32)
    obd = np.zeros((128, 64), np.float32)
    for half in (0, 64):
        for j in range(4):
            obd[half + 16 * j:half + 16 * j + 16, 16 * j:16 * j + 16] = 1.0
    obd_dev = obd.astype(ml_dtypes.bfloat16)
    return {'wq': wq_dev, 'w2c': w2c_dev, 'wp': wp_dev, 'bnb': bnb_dev, 'obd': obd_dev}


def make_in_maps(inputs):
    x = np.asarray(inputs['x'], dtype=np.float32)
    wdev = host_weights(
        np.asarray(inputs['w_qkv'], np.float32), np.asarray(inputs['w_dw'], np.float32),
        np.asarray(inputs['w_pw'], np.float32), np.asarray(inputs['w_proj'], np.float32),
        np.asarray(inputs['bn_gamma'], np.float32), np.asarray(inputs['bn_beta'], np.float32),
        np.asarray(inputs['bn_mean'], np.float32), np.asarray(inputs['bn_var'], np.float32))
    in_maps = []
    for b in range(B):
        xb = x[b].reshape(CIN, S).reshape(NKT, 128, S).transpose(1, 0, 2)
        in_maps.append({'x16': np.ascontiguousarray(xb).astype(ml_dtypes.bfloat16), **wdev})
    return in_maps


def kernel(x, w_qkv, w_dw, w_pw, w_proj, bn_gamma, bn_beta, bn_mean, bn_var):
    if 'nc' not in _CACHED:
        _CACHED['nc'] = build_program()
    nc = _CACHED['nc']
    in_maps = make_in_maps(dict(
        x=x, w_qkv=w_qkv, w_dw=w_dw, w_pw=w_pw, w_proj=w_proj,
        bn_gamma=bn_gamma, bn_beta=bn_beta, bn_mean=bn_mean, bn_var=bn_var))
    res = run_bass_kernel_spmd(nc, in_maps, list(range(B)))
    y = np.stack([res.results[b]['y_b'].reshape(512, H, W) for b in range(B)])
    return y.astype(np.float32)
